# revision 85
# baseline (speedup 1.0000x reference)
"""Longformer-style windowed self-attention for TRN2, 8-core SPMD.

Sharding: 24 (batch, head) pairs -> 3 heads per core (core c gets batch c//4,
heads (c%4)*3 .. +3). Each core computes QKV projections for its head slice,
windowed attention (query superblocks of 512, window +-256), and writes its
[4096, 192] output channel slice. Host gathers slices into the full
[2, 4096, 768] output.

All matmul inputs are bf16 (psum accumulation fp32). Scores are computed
transposed ([keys, queries]) in 512-query superblocks: the 8-chunk key
window is trimmed so chunk i only streams the queries it can reach
(extents 128/256/384/512/512/384/256/128 = 2560 columns per head instead
of 4096), packed into two 1280-column psum pieces with no bank-crossing
matmuls, one contiguous exp (Activation) per piece, and one or two strided
diagonal-mask multiplies (GpSimd). Softmax renormalization reduces over
the partition dim via a ones-column appended to V in the PV matmul;
results are PE-transposed back and scaled by reciprocal row sums.

Schedule: a software pipeline interleaves projection tiles, score heads,
and - deferred by PVLAG superblocks - PV heads and epilogues, so the
deferred PV work fills the tensor engine in the post-projection phase
where scores would otherwise lockstep with the Act engine's exp drain
(the two 3-bank score psum slots only free once exp'd). Tail-phase
epilogue copies/scales run on the Act engine, which has no exps left to
do there.
"""

import sys

for _p in ("/opt/trn_rl_repo", "/opt/pypackages"):
    if _p not in sys.path:
        sys.path.append(_p)

import numpy as np
import ml_dtypes
from contextlib import ExitStack

import concourse.bass as bass
import concourse.bacc as bacc
import concourse.mybir as mybir
import concourse.tile as tile
from concourse.bass_utils import run_bass_kernel_spmd

F32 = mybir.dt.float32
R32 = mybir.dt.float32r
BF16 = mybir.dt.bfloat16
EXP = mybir.ActivationFunctionType.Exp
BF = ml_dtypes.bfloat16

B, S, D = 2, 4096, 768
H, DH = 12, 64
W = 256                 # one-sided window / query block size
NB = S // W             # 16 query blocks
NKC = S // 128          # 32 key chunks of 128
HPC = 3                 # heads per core
N_CORES = 8


NSB = S // 512          # 8 query superblocks of 512

# psum column of chunk i within its piece (piece 0: i<=3, piece 1: i>=4)
_SB_COL = {2: 0, 0: 384, 3: 512, 1: 1024, 4: 0, 5: 512, 7: 896, 6: 1024}


def _sb_chunks(s):
    """Superblock s covers queries [512s, 512s+512); its key window is the
    8 chunks m = 4s-2 .. 4s+5 (chunk position i = m - 4s + 2). Chunk i is
    valid for superblock-relative queries [max(0, 128(i-4)), min(512,
    128(i+1))) — extents 128/256/384/512/512/384/256/128. Left chunks
    (i<=3) are diagonal-masked on the last 128 columns of their extent
    (keep j <= r), right chunks (i>=4) on the first 128 (keep j >= r).

    Returns [(i, m, piece, col, width, qlo)].
    """
    out = []
    for i in range(8):
        m = 4 * s - 2 + i
        if not (0 <= m < NKC):
            continue
        qlo = max(0, 128 * (i - 4))
        qhi = min(512, 128 * (i + 1))
        out.append((i, m, i // 4, _SB_COL[i], qhi - qlo, qlo))
    return out


def _merge_ranges(ivals):
    """Merge sorted [start, end) col intervals into contiguous runs."""
    ivals = sorted(ivals)
    out = [list(ivals[0])]
    for a, b_ in ivals[1:]:
        if a == out[-1][1]:
            out[-1][1] = b_
        else:
            out.append([a, b_])
    return [(a, b_ - a) for a, b_ in out]


def build_program(has_bias, has_kmask):
    nc = bacc.Bacc("TRN2", target_bir_lowering=False, debug=False,
                   num_devices=N_CORES)
    hsT_d = nc.declare_dram_parameter("hsT", [D, S], BF16, isOutput=False)
    wqk_d = nc.declare_dram_parameter("wqk", [D, 384], BF16, isOutput=False)
    wv_d = nc.declare_dram_parameter("wv", [D, 192], BF16, isOutput=False)
    msk_d = nc.declare_dram_parameter("masks", [128, 512], BF16, isOutput=False)
    idn_d = nc.declare_dram_parameter("ident", [128, 128], BF16, isOutput=False)
    if has_bias:
        bqk_d = nc.declare_dram_parameter("bqk", [1, 384], BF16, isOutput=False)
        bv_d = nc.declare_dram_parameter("bv", [1, 192], BF16, isOutput=False)
    if has_kmask:
        kpad_d = nc.declare_dram_parameter("kpad", [128, NKC], F32, isOutput=False)
        qpad_d = nc.declare_dram_parameter("qpad", [128, NKC], F32, isOutput=False)
    out_d = nc.declare_dram_parameter("out", [S, HPC * DH], F32, isOutput=True)

    with tile.TileContext(nc) as tc, ExitStack() as ctx:
        const_p = ctx.enter_context(tc.tile_pool(name="const", bufs=1))
        hst_p = ctx.enter_context(tc.tile_pool(name="hst", bufs=3))
        qkt_p = ctx.enter_context(tc.tile_pool(name="qkt", bufs=1))
        vall_p = ctx.enter_context(tc.tile_pool(name="vall", bufs=1))
        pt_p = ctx.enter_context(tc.tile_pool(name="pt", bufs=24))
        wk_p = ctx.enter_context(tc.tile_pool(name="wk", bufs=16))
        ps_p = ctx.enter_context(tc.tile_pool(name="ps", bufs=2, space="PSUM"))
        sm_p = ctx.enter_context(tc.tile_pool(name="sm", bufs=2, space="PSUM"))

        # ---- constants / weights ----
        wqk_sb = const_p.tile([128, 6, 384], BF16)
        wv_sb = const_p.tile([128, 6, 192], BF16)
        msk_sb = const_p.tile([128, 512], BF16)
        idn_sb = const_p.tile([128, 128], BF16)
        nc.sync.dma_start(idn_sb[:], idn_d[:, :])
        if has_bias:
            bqk_sb = const_p.tile([1, 384], BF16)
            nc.sync.dma_start(bqk_sb[:], bqk_d[:, :])
            bv_sb = const_p.tile([1, 192], BF16)
            nc.sync.dma_start(bv_sb[:], bv_d[:, :])
            ones_sb = const_p.tile([1, 512], BF16)
            nc.vector.memset(ones_sb[:], 1.0)
        if has_kmask:
            kpad_sb = const_p.tile([128, NKC], F32)
            nc.sync.dma_start(kpad_sb[:], kpad_d[:, :])
            qpad_sb = const_p.tile([128, NKC], F32)
            nc.sync.dma_start(qpad_sb[:], qpad_d[:, :])

        # qT/kT for head pair (A,B): A on partitions 0:64, B on 64:128.
        # Head C: qkt_c holds qC on 0:64 / kC on 64:128; qkt_c2[0:64] is a
        # DMA-replicated copy of kC so both score operands sit on 0:64.
        qt_ab = qkt_p.tile([128, S], BF16)
        kt_ab = qkt_p.tile([128, S], BF16)
        qkt_c = qkt_p.tile([128, S], BF16)
        qkt_c2 = qkt_p.tile([64, S], BF16)
        # v in [s, dh] layout: [128, key-chunk, (vA|1|vB|1|vC|1)]
        vall = vall_p.tile([128, NKC, 195], BF16)
        ones_cols = vall[:].rearrange("p m (h x) -> p m h x", h=3)[:, :, :, 64:65]
        nc.vector.memset(ones_cols, 1.0)

        hst_tiles = {}

        def emit_proj_dma(t, split=False):
            hst = hst_p.tile([128, 6, 512], BF16)
            hst_tiles[t] = hst
            s0 = 512 * t
            src = hsT_d[:].rearrange("(c p) s -> p c s", p=128)[:, :, s0 : s0 + 512]
            if split:
                # split on the contraction-chunk dim: the projection's c-loop
                # consumes chunks in order, so matmuls start after the first
                # piece lands.
                nc.sync.dma_start(hst[:, 0:2, :], src[:, 0:2, :])
                nc.sync.dma_start(hst[:, 2:6, :], src[:, 2:6, :])
            else:
                nc.sync.dma_start(hst[:], src)

        def emit_proj_qk(t):
            s0 = 512 * t
            hst = hst_tiles[t]
            # q/k projections: 3 pair-matmuls of M=128 -> [qA|qB], [kA|kB],
            # [qC|kC]
            for j in range(3):
                pp = sm_p.tile([128, 512], F32, space="PSUM", tag="sm")
                for c in range(6):
                    nc.tensor.matmul(
                        pp[:],
                        wqk_sb[:, c, 128 * j : 128 * j + 128],
                        hst[:, c, :],
                        start=(c == 0),
                        stop=(c == 5 and not has_bias),
                    )
                if has_bias:
                    nc.tensor.matmul(
                        pp[:],
                        bqk_sb[0:1, 128 * j : 128 * j + 128],
                        ones_sb[0:1, :],
                        start=False,
                        stop=True,
                    )
                dst = (qt_ab, kt_ab, qkt_c)[j]
                nc.vector.tensor_copy(dst[:, s0 : s0 + 512], pp[:])
            nc.sync.dma_start(qkt_c2[:, s0 : s0 + 512], qkt_c[64:128, s0 : s0 + 512])

        def emit_proj_v(t):
            s0 = 512 * t
            hst = hst_tiles.pop(t)
            # v projection: 4 s-subtiles of 128, packed two per PSUM tile
            for mm0 in (0, 2):
                m = 4 * t + mm0
                pv = sm_p.tile([128, 512], F32, space="PSUM", tag="sm")
                for half, mm in enumerate((mm0, mm0 + 1)):
                    for c in range(6):
                        nc.tensor.matmul(
                            pv[:, 256 * half : 256 * half + 192],
                            hst[:, c, 128 * mm : 128 * mm + 128],
                            wv_sb[:, c, :],
                            start=(c == 0),
                            stop=(c == 5 and not has_bias),
                        )
                    if has_bias:
                        nc.tensor.matmul(
                            pv[:, 256 * half : 256 * half + 192],
                            ones_sb[0:1, 0:128],
                            bv_sb[0:1, :],
                            start=False,
                            stop=True,
                        )
                dst = vall[:, m : m + 2, :].rearrange(
                    "p m (h x) -> p m h x", h=3
                )[:, :, :, 0:64]
                src = pv[:].rearrange("p (m x) -> p m x", m=2)[
                    :, :, 0:192
                ].rearrange("p m (h x) -> p m h x", h=3)
                nc.vector.tensor_copy(dst, src)

        def emit_mask(pt, in_off, nreg, stride, msk_off):
            """pt[:, in_off + k*stride : +128] *= msk[:, msk_off + k*128]
            for k in range(nreg), as one strided TensorTensor."""
            if nreg == 1:
                in_ap = pt[:, in_off : in_off + 128]
                mk_ap = msk_sb[:, msk_off : msk_off + 128]
            else:
                ln = stride * (nreg - 1) + 128
                in_ap = pt[:, in_off : in_off + ln].rearrange(
                    "p (a x) -> p a x", x=128
                )[:, :: stride // 128, :]
                mk_ap = msk_sb[:, msk_off : msk_off + 128 * nreg].rearrange(
                    "p (a x) -> p a x", x=128
                )
            nc.gpsimd.tensor_mul(in_ap, in_ap, mk_ap)

        # per-superblock state flowing scores -> PV -> epilogue
        blk = {}

        def emit_scores_head(s, h):
            q0 = 512 * s
            chunks = _sb_chunks(s)
            if h == 0:
                kt, qt, p0 = kt_ab, qt_ab, 0
            elif h == 1:
                kt, qt, p0 = kt_ab, qt_ab, 64
            else:
                kt, qt, p0 = qkt_c2, qkt_c, 0
            hpt = []
            blk.setdefault(s, {"pts": [], "ots": []})["pts"].append(hpt)
            if True:
                for piece in range(2):
                    pc = [c for c in chunks if c[2] == piece]
                    ps = ps_p.tile([128, 1536], F32, space="PSUM", tag="ps")
                    for i, m, _, col, w_, qlo in pc:
                        nc.tensor.matmul(
                            ps[:, col : col + w_],
                            kt[p0 : p0 + 64, 128 * m : 128 * m + 128],
                            qt[p0 : p0 + 64, q0 + qlo : q0 + qlo + w_],
                            start=True,
                            stop=True,
                            tile_position=(p0, 0),
                        )
                    pt = pt_p.tile([128, 1536], BF16, tag="pt")
                    for a, ln in _merge_ranges(
                        [(col, col + w_) for _, _, _, col, w_, _ in pc]
                    ):
                        nc.scalar.activation(pt[:, a : a + ln], ps[:, a : a + ln], EXP)
                    # diagonal masks: left chunks (i<=3) keep j <= r on the
                    # last 128 cols of their extent, right chunks keep
                    # j >= r on the first 128.
                    moffs = sorted(
                        (col + w_ - 128) if i <= 3 else col
                        for i, _, _, col, w_, _ in pc
                    )
                    mbase = 0 if piece == 0 else 256
                    k = 0
                    while k < len(moffs):
                        nreg = 1
                        while (
                            k + nreg < len(moffs)
                            and moffs[k + nreg] - moffs[k + nreg - 1]
                            == moffs[k + 1] - moffs[k]
                        ):
                            nreg += 1
                        stride = moffs[k + 1] - moffs[k] if nreg > 1 else 128
                        emit_mask(pt, moffs[k], nreg, stride, mbase)
                        k += nreg
                    if has_kmask:
                        for i, m, _, col, w_, qlo in pc:
                            nc.vector.tensor_scalar_mul(
                                pt[:, col : col + w_],
                                pt[:, col : col + w_],
                                kpad_sb[:, m : m + 1],
                            )
                    hpt.append((pt, pc))

        def emit_pv_head(s, h):
            # i3 (always full 512-wide) starts the psum group, i4 (also
            # full) stops it; partial-extent chunks accumulate between.
            st = blk[s]
            bych = {c[0]: (pc, c) for pc, ch in st["pts"][h] for c in ch}
            order = [3] + [i for i in (0, 1, 2, 5, 6, 7) if i in bych] + [4]
            pv = sm_p.tile([128, 512], F32, space="PSUM", tag="sm")
            for oi, i in enumerate(order):
                pt, (_, m, _, col, w_, qlo) = bych[i]
                nc.tensor.matmul(
                    pv[0:65, qlo : qlo + w_],
                    vall[:, m, 65 * h : 65 * h + 65],
                    pt[:, col : col + w_],
                    start=(oi == 0),
                    stop=(oi == len(order) - 1),
                    skip_group_check=True,
                )
            ot = wk_p.tile([65, 512], BF16, name=f"ot{h}")
            # tail-phase epilogues lean on the Act engine, which has no
            # exps left to run there
            if s >= NSB - 3 and h >= 1:
                nc.scalar.copy(ot[:], pv[0:65, :])
            else:
                nc.vector.tensor_copy(ot[:], pv[0:65, :])
            st["ots"].append(ot)

        def emit_epi_head(s, h):
            # Epilogue: transpose head h's [65, 512] into trp bank h
            # (66-spaced query-quarters; col 64 of each group is the softmax
            # denominator), then scale by the reciprocal row sums.
            st = blk[s]
            if "trp" not in st:
                st["trp"] = ps_p.tile(
                    [128, 1536], BF16, space="PSUM", tag="ps", name="trp"
                )
                st["rec"] = wk_p.tile([128, 16], F32, name="rec")
                st["osbs"] = [
                    wk_p.tile([128, 192], F32, name="osb") for _ in range(4)
                ]
            trp, rec, osbs = st["trp"], st["rec"], st["osbs"]
            for g in range(4):
                nc.tensor.transpose(
                    trp[:, 512 * h + 66 * g : 512 * h + 66 * g + 65],
                    st["ots"][h][0:65, 128 * g : 128 * g + 128],
                    idn_sb[0:65, 0:65],
                )
            dcol = trp[:, 512 * h : 512 * h + 264].rearrange(
                "p (i x) -> p i x", x=66
            )[:, :, 64:65]
            nc.vector.reciprocal(
                rec[:, 4 * h : 4 * h + 4].rearrange("p (i x) -> p i x", x=1),
                dcol,
            )
            for g in range(4):
                args = (
                    osbs[g][:, 64 * h : 64 * h + 64],
                    trp[:, 512 * h + 66 * g : 512 * h + 66 * g + 64],
                    rec[:, 4 * h + g : 4 * h + g + 1],
                )
                if s >= NSB - 3 and h >= 1:
                    nc.scalar.mul(*args)
                else:
                    nc.vector.tensor_scalar_mul(*args)

        def emit_epi_out(s):
            q0 = 512 * s
            st = blk.pop(s)
            for g in range(4):
                if has_kmask:
                    nc.vector.tensor_scalar_mul(
                        st["osbs"][g][:], st["osbs"][g][:],
                        qpad_sb[:, 4 * s + g : 4 * s + g + 1],
                    )
                nc.sync.dma_start(
                    out_d[q0 + 128 * g : q0 + 128 * g + 128, 0:192],
                    st["osbs"][g][:],
                )

        # Software pipeline: scores(s) on PE while exp/mask(s-1) drain on
        # Act/Pool, then PV+epilogue(s-1); projection work interleaves.
        # scores(i) needs qk through tile i+1; pv(i-1) needs v through tile
        # i. DMA order front-loads what the first matmuls need: wqk, hst(0),
        # then the rest of the constants.
        wqk_src = wqk_d[:].rearrange("(c p) n -> p c n", p=128)
        nc.sync.dma_start(wqk_sb[:, 0:2, :], wqk_src[:, 0:2, :])
        emit_proj_dma(0, split=True)
        nc.sync.dma_start(wqk_sb[:, 2:6, :], wqk_src[:, 2:6, :])
        nc.sync.dma_start(wv_sb[:], wv_d[:].rearrange("(c p) n -> p c n", p=128))
        emit_proj_qk(0)
        nc.sync.dma_start(msk_sb[:], msk_d[:, :])
        emit_proj_dma(1)
        emit_proj_qk(1)
        emit_proj_v(0)
        emit_proj_dma(2)
        # PV+epilogue trail scores by PVLAG+1 superblocks: the deferred PV
        # work fills PE during the post-projection iterations where scores
        # would otherwise lockstep with the Act engine's exp drain (psum
        # score slots only free once exp'd). trp(p) must allocate after
        # scores(i)'s psum tiles or a later score tile would evict it
        # before its readers.
        PVLAG = 2
        for i in range(NSB + PVLAG + 1):
            p = i - PVLAG - 1
            if i + 3 <= 7:
                emit_proj_dma(i + 3)
            # tail-assist threshold: epilogues running after the last exps
            # drain can borrow the Act engine
            if i + 2 <= 7:
                emit_proj_qk(i + 2)
            if i == NSB + PVLAG:
                # last iteration: interleave the epilogue chains between the
                # remaining PV heads so the end chain starts sooner
                emit_pv_head(p, 0)
                emit_pv_head(p, 1)
                emit_epi_head(p, 0)
                emit_pv_head(p, 2)
                emit_epi_head(p, 1)
                emit_epi_head(p, 2)
                emit_epi_out(p)
            else:
                for h in range(3):
                    if i < NSB:
                        emit_scores_head(i, h)
                    if p >= 0:
                        emit_pv_head(p, h)
                if p >= 0:
                    for h in range(3):
                        emit_epi_head(p, h)
                    emit_epi_out(p)
            if 1 <= i + 1 <= 7:
                emit_proj_v(i + 1)

    nc.compile()
    return nc


_prog_cache = {}


def _get_program(has_bias, has_kmask):
    key = (has_bias, has_kmask)
    if key not in _prog_cache:
        _prog_cache[key] = build_program(has_bias, has_kmask)
    return _prog_cache[key]


def _band_masks():
    """[mL | mL | mR | mR] multiplicative diagonal masks, [128, 512].

    In [key-row r, query-col j] space: mL keeps j <= r (left window edge),
    mR keeps j >= r (right edge); each appears twice so 2-region strided
    mask ops can read consecutive 128-col groups.
    """
    r = np.arange(128)[:, None]
    q = np.arange(128)[None, :]
    mL = (q <= r).astype(np.float32)
    mR = (q >= r).astype(np.float32)
    return np.concatenate([mL, mL, mR, mR], axis=1)


def kernel(hidden_states, attention_mask, Wq, bq, Wk, bk, Wv, bv, _res=[None]):
    hidden_states = np.asarray(hidden_states, np.float32)
    attention_mask = np.asarray(attention_mask, np.float32)
    Wq, Wk, Wv = (np.asarray(w, np.float32) for w in (Wq, Wk, Wv))
    bq, bk, bv = (np.asarray(b_, np.float32) for b_ in (bq, bk, bv))

    scale = 1.0 / np.sqrt(DH)
    has_bias = bool(np.any(bq) or np.any(bk) or np.any(bv))
    has_kmask = bool(np.any(attention_mask < 0))

    hsT = [np.ascontiguousarray(hidden_states[b].T).astype(BF) for b in range(B)]
    masks = _band_masks().astype(BF)
    ident = np.eye(128, dtype=np.float32).astype(BF)
    ident = np.eye(128, dtype=np.float32).astype(BF)
    masked = attention_mask < 0  # [B, S]

    in_maps = []
    for core in range(N_CORES):
        b, h0 = core // 4, (core % 4) * HPC
        sl = slice(h0 * DH, (h0 + HPC) * DH)
        wq = Wq[:, sl] * scale
        wk = Wk[:, sl]
        wqk = np.concatenate(
            [wq[:, 0:128], wk[:, 0:128], wq[:, 128:192], wk[:, 128:192]], axis=1
        )
        m = {
            "hsT": hsT[b],
            "wqk": np.ascontiguousarray(wqk).astype(BF),
            "wv": np.ascontiguousarray(Wv[:, sl]).astype(BF),
            "masks": masks,
            "ident": ident,
        }
        if has_bias:
            bq_s = bq[sl] * scale
            bk_s = bk[sl]
            m["bqk"] = np.concatenate(
                [bq_s[0:128], bk_s[0:128], bq_s[128:192], bk_s[128:192]]
            ).reshape(1, 384).astype(BF)
            m["bv"] = bv[sl].reshape(1, 192).astype(BF)
        if has_kmask:
            keep = (~masked[b]).astype(np.float32).reshape(NKC, 128).T
            m["kpad"] = np.ascontiguousarray(keep)
            m["qpad"] = np.ascontiguousarray(keep)
        in_maps.append(m)

    nc = _get_program(has_bias, has_kmask)
    res = run_bass_kernel_spmd(nc, in_maps, list(range(N_CORES)))
    _res[0] = res

    out = np.empty((B, S, D), np.float32)
    for core in range(N_CORES):
        b, h0 = core // 4, (core % 4) * HPC
        out[b, :, h0 * DH : (h0 + HPC) * DH] = res.results[core]["out"]
    return out


# revision 99
# speedup vs baseline: 1.0360x; 1.0360x over previous
"""Longformer-style windowed self-attention for TRN2, 8-core SPMD.

Sharding: 24 (batch, head) pairs -> 3 heads per core (core c gets batch c//4,
heads (c%4)*3 .. +3). Each core computes QKV projections for its head slice,
windowed attention (query superblocks of 512, window +-256), and writes its
[4096, 192] output channel slice. Host gathers slices into the full
[2, 4096, 768] output.

All matmul inputs are bf16 (psum accumulation fp32). Scores are computed
transposed ([keys, queries]) in 512-query superblocks: the 8-chunk key
window is trimmed so chunk i only streams the queries it can reach
(extents 128/256/384/512/512/384/256/128 = 2560 columns per head instead
of 4096), packed into two 1280-column psum pieces with no bank-crossing
matmuls, one contiguous exp (Activation) per piece, and one or two strided
diagonal-mask multiplies (GpSimd). Softmax renormalization reduces over
the partition dim via a ones-column appended to V in the PV matmul;
results are PE-transposed back and scaled by reciprocal row sums.

Schedule: a software pipeline interleaves projection tiles, score heads,
and - deferred by PVLAG superblocks - PV heads and epilogues, so the
deferred PV work fills the tensor engine in the post-projection phase
where scores would otherwise lockstep with the Act engine's exp drain
(the two 3-bank score psum slots only free once exp'd). Tail-phase
epilogue copies/scales run on the Act engine, which has no exps left to
do there.
"""

import sys

for _p in ("/opt/trn_rl_repo", "/opt/pypackages"):
    if _p not in sys.path:
        sys.path.append(_p)

import numpy as np
import ml_dtypes
from contextlib import ExitStack

import concourse.bass as bass
import concourse.bacc as bacc
import concourse.mybir as mybir
import concourse.tile as tile
from concourse.bass_utils import run_bass_kernel_spmd

F32 = mybir.dt.float32
R32 = mybir.dt.float32r
BF16 = mybir.dt.bfloat16
EXP = mybir.ActivationFunctionType.Exp
BF = ml_dtypes.bfloat16

B, S, D = 2, 4096, 768
H, DH = 12, 64
W = 256                 # one-sided window / query block size
NB = S // W             # 16 query blocks
NKC = S // 128          # 32 key chunks of 128
HPC = 3                 # heads per core
N_CORES = 8


NSB = S // 512          # 8 query superblocks of 512

# psum column of chunk i within its piece (piece 0: i<=3, piece 1: i>=4)
_SB_COL = {2: 0, 0: 384, 3: 512, 1: 1024, 4: 0, 5: 512, 7: 896, 6: 1024}


def _sb_chunks(s):
    """Superblock s covers queries [512s, 512s+512); its key window is the
    8 chunks m = 4s-2 .. 4s+5 (chunk position i = m - 4s + 2). Chunk i is
    valid for superblock-relative queries [max(0, 128(i-4)), min(512,
    128(i+1))) — extents 128/256/384/512/512/384/256/128. Left chunks
    (i<=3) are diagonal-masked on the last 128 columns of their extent
    (keep j <= r), right chunks (i>=4) on the first 128 (keep j >= r).

    Returns [(i, m, piece, col, width, qlo)].
    """
    out = []
    for i in range(8):
        m = 4 * s - 2 + i
        if not (0 <= m < NKC):
            continue
        qlo = max(0, 128 * (i - 4))
        qhi = min(512, 128 * (i + 1))
        out.append((i, m, i // 4, _SB_COL[i], qhi - qlo, qlo))
    return out


def _merge_ranges(ivals):
    """Merge sorted [start, end) col intervals into contiguous runs."""
    ivals = sorted(ivals)
    out = [list(ivals[0])]
    for a, b_ in ivals[1:]:
        if a == out[-1][1]:
            out[-1][1] = b_
        else:
            out.append([a, b_])
    return [(a, b_ - a) for a, b_ in out]


def build_program(has_bias, has_kmask):
    nc = bacc.Bacc("TRN2", target_bir_lowering=False, debug=False,
                   num_devices=N_CORES)
    hsT_d = nc.declare_dram_parameter("hsT", [D, S], BF16, isOutput=False)
    wqk_d = nc.declare_dram_parameter("wqk", [D, 384], BF16, isOutput=False)
    wv_d = nc.declare_dram_parameter("wv", [D, 192], BF16, isOutput=False)
    msk_d = nc.declare_dram_parameter("masks", [128, 512], BF16, isOutput=False)
    idn_d = nc.declare_dram_parameter("ident", [128, 128], BF16, isOutput=False)
    if has_bias:
        bqk_d = nc.declare_dram_parameter("bqk", [1, 384], BF16, isOutput=False)
        bv_d = nc.declare_dram_parameter("bv", [1, 192], BF16, isOutput=False)
    if has_kmask:
        kpad_d = nc.declare_dram_parameter("kpad", [128, NKC], F32, isOutput=False)
        qpad_d = nc.declare_dram_parameter("qpad", [128, NKC], F32, isOutput=False)
    out_d = nc.declare_dram_parameter("out", [S, HPC * DH], F32, isOutput=True)

    with tile.TileContext(nc) as tc, ExitStack() as ctx:
        const_p = ctx.enter_context(tc.tile_pool(name="const", bufs=1))
        hst_p = ctx.enter_context(tc.tile_pool(name="hst", bufs=3))
        qkt_p = ctx.enter_context(tc.tile_pool(name="qkt", bufs=1))
        vall_p = ctx.enter_context(tc.tile_pool(name="vall", bufs=1))
        pt_p = ctx.enter_context(tc.tile_pool(name="pt", bufs=24))
        wk_p = ctx.enter_context(tc.tile_pool(name="wk", bufs=16))
        ps_p = ctx.enter_context(tc.tile_pool(name="ps", bufs=2, space="PSUM"))
        sm_p = ctx.enter_context(tc.tile_pool(name="sm", bufs=2, space="PSUM"))

        # ---- constants / weights ----
        wqk_sb = const_p.tile([128, 6, 384], BF16)
        wv_sb = const_p.tile([128, 6, 192], BF16)
        msk_sb = const_p.tile([128, 512], BF16)
        idn_sb = const_p.tile([128, 128], BF16)
        nc.sync.dma_start(idn_sb[:], idn_d[:, :])
        if has_bias:
            bqk_sb = const_p.tile([1, 384], BF16)
            nc.sync.dma_start(bqk_sb[:], bqk_d[:, :])
            bv_sb = const_p.tile([1, 192], BF16)
            nc.sync.dma_start(bv_sb[:], bv_d[:, :])
            ones_sb = const_p.tile([1, 512], BF16)
            nc.vector.memset(ones_sb[:], 1.0)
        if has_kmask:
            kpad_sb = const_p.tile([128, NKC], F32)
            nc.sync.dma_start(kpad_sb[:], kpad_d[:, :])
            qpad_sb = const_p.tile([128, NKC], F32)
            nc.sync.dma_start(qpad_sb[:], qpad_d[:, :])

        # qT/kT for head pair (A,B): A on partitions 0:64, B on 64:128.
        # Head C: qkt_c holds qC on 0:64 / kC on 64:128; qkt_c2[0:64] is a
        # DMA-replicated copy of kC so both score operands sit on 0:64.
        qt_ab = qkt_p.tile([128, S], BF16)
        kt_ab = qkt_p.tile([128, S], BF16)
        qkt_c = qkt_p.tile([128, S], BF16)
        qkt_c2 = qkt_p.tile([64, S], BF16)
        # v in [s, dh] layout: [128, key-chunk, (vA|1|vB|1|vC|1)]
        vall = vall_p.tile([128, NKC, 195], BF16)
        ones_cols = vall[:].rearrange("p m (h x) -> p m h x", h=3)[:, :, :, 64:65]
        nc.vector.memset(ones_cols, 1.0)

        hst_tiles = {}

        def emit_proj_dma(t, split=False):
            hst = hst_p.tile([128, 6, 512], BF16)
            hst_tiles[t] = hst
            s0 = 512 * t
            src = hsT_d[:].rearrange("(c p) s -> p c s", p=128)[:, :, s0 : s0 + 512]
            if split:
                # split on the contraction-chunk dim: the projection's c-loop
                # consumes chunks in order, so matmuls start after the first
                # piece lands.
                nc.sync.dma_start(hst[:, 0:2, :], src[:, 0:2, :])
                nc.sync.dma_start(hst[:, 2:6, :], src[:, 2:6, :])
            else:
                nc.sync.dma_start(hst[:], src)

        def emit_proj_qk(t):
            s0 = 512 * t
            hst = hst_tiles[t]
            # q/k projections: 3 pair-matmuls of M=128 -> [qA|qB], [kA|kB],
            # [qC|kC]
            for j in range(3):
                pp = sm_p.tile([128, 512], F32, space="PSUM", tag="sm")
                for c in range(6):
                    nc.tensor.matmul(
                        pp[:],
                        wqk_sb[:, c, 128 * j : 128 * j + 128],
                        hst[:, c, :],
                        start=(c == 0),
                        stop=(c == 5 and not has_bias),
                    )
                if has_bias:
                    nc.tensor.matmul(
                        pp[:],
                        bqk_sb[0:1, 128 * j : 128 * j + 128],
                        ones_sb[0:1, :],
                        start=False,
                        stop=True,
                    )
                dst = (qt_ab, kt_ab, qkt_c)[j]
                nc.vector.tensor_copy(dst[:, s0 : s0 + 512], pp[:])
            nc.sync.dma_start(qkt_c2[:, s0 : s0 + 512], qkt_c[64:128, s0 : s0 + 512])

        def emit_proj_v(t):
            s0 = 512 * t
            hst = hst_tiles.pop(t)
            # v projection: 4 s-subtiles of 128, packed two per PSUM tile
            for mm0 in (0, 2):
                m = 4 * t + mm0
                pv = sm_p.tile([128, 512], F32, space="PSUM", tag="sm")
                for half, mm in enumerate((mm0, mm0 + 1)):
                    for c in range(6):
                        nc.tensor.matmul(
                            pv[:, 256 * half : 256 * half + 192],
                            hst[:, c, 128 * mm : 128 * mm + 128],
                            wv_sb[:, c, :],
                            start=(c == 0),
                            stop=(c == 5 and not has_bias),
                        )
                    if has_bias:
                        nc.tensor.matmul(
                            pv[:, 256 * half : 256 * half + 192],
                            ones_sb[0:1, 0:128],
                            bv_sb[0:1, :],
                            start=False,
                            stop=True,
                        )
                dst = vall[:, m : m + 2, :].rearrange(
                    "p m (h x) -> p m h x", h=3
                )[:, :, :, 0:64]
                src = pv[:].rearrange("p (m x) -> p m x", m=2)[
                    :, :, 0:192
                ].rearrange("p m (h x) -> p m h x", h=3)
                nc.vector.tensor_copy(dst, src)

        def emit_mask(pt, in_off, nreg, stride, msk_off):
            """pt[:, in_off + k*stride : +128] *= msk[:, msk_off + k*128]
            for k in range(nreg), as one strided TensorTensor."""
            if nreg == 1:
                in_ap = pt[:, in_off : in_off + 128]
                mk_ap = msk_sb[:, msk_off : msk_off + 128]
            else:
                ln = stride * (nreg - 1) + 128
                in_ap = pt[:, in_off : in_off + ln].rearrange(
                    "p (a x) -> p a x", x=128
                )[:, :: stride // 128, :]
                mk_ap = msk_sb[:, msk_off : msk_off + 128 * nreg].rearrange(
                    "p (a x) -> p a x", x=128
                )
            nc.gpsimd.tensor_mul(in_ap, in_ap, mk_ap)

        # per-superblock state flowing scores -> PV -> epilogue
        blk = {}

        def emit_scores_head(s, h):
            q0 = 512 * s
            chunks = _sb_chunks(s)
            if h == 0:
                kt, qt, p0 = kt_ab, qt_ab, 0
            elif h == 1:
                kt, qt, p0 = kt_ab, qt_ab, 64
            else:
                kt, qt, p0 = qkt_c2, qkt_c, 0
            hpt = []
            blk.setdefault(s, {"pts": [], "ots": []})["pts"].append(hpt)
            if True:
                for piece in range(2):
                    pc = [c for c in chunks if c[2] == piece]
                    ps = ps_p.tile([128, 1536], F32, space="PSUM", tag="ps")
                    for i, m, _, col, w_, qlo in pc:
                        nc.tensor.matmul(
                            ps[:, col : col + w_],
                            kt[p0 : p0 + 64, 128 * m : 128 * m + 128],
                            qt[p0 : p0 + 64, q0 + qlo : q0 + qlo + w_],
                            start=True,
                            stop=True,
                            tile_position=(p0, 0),
                        )
                    pt = pt_p.tile([128, 1536], BF16, tag="pt")
                    for a, ln in _merge_ranges(
                        [(col, col + w_) for _, _, _, col, w_, _ in pc]
                    ):
                        nc.scalar.activation(pt[:, a : a + ln], ps[:, a : a + ln], EXP)
                    # diagonal masks: left chunks (i<=3) keep j <= r on the
                    # last 128 cols of their extent, right chunks keep
                    # j >= r on the first 128.
                    moffs = sorted(
                        (col + w_ - 128) if i <= 3 else col
                        for i, _, _, col, w_, _ in pc
                    )
                    mbase = 0 if piece == 0 else 256
                    k = 0
                    while k < len(moffs):
                        nreg = 1
                        while (
                            k + nreg < len(moffs)
                            and moffs[k + nreg] - moffs[k + nreg - 1]
                            == moffs[k + 1] - moffs[k]
                        ):
                            nreg += 1
                        stride = moffs[k + 1] - moffs[k] if nreg > 1 else 128
                        emit_mask(pt, moffs[k], nreg, stride, mbase)
                        k += nreg
                    if has_kmask:
                        for i, m, _, col, w_, qlo in pc:
                            nc.vector.tensor_scalar_mul(
                                pt[:, col : col + w_],
                                pt[:, col : col + w_],
                                kpad_sb[:, m : m + 1],
                            )
                    hpt.append((pt, pc))

        def emit_pv_head(s, h):
            # i3 (always full 512-wide) starts the psum group, i4 (also
            # full) stops it; partial-extent chunks accumulate between.
            st = blk[s]
            bych = {c[0]: (pc, c) for pc, ch in st["pts"][h] for c in ch}
            order = [3] + [i for i in (0, 1, 2, 5, 6, 7) if i in bych] + [4]
            pv = sm_p.tile([128, 512], F32, space="PSUM", tag="sm")
            for oi, i in enumerate(order):
                pt, (_, m, _, col, w_, qlo) = bych[i]
                nc.tensor.matmul(
                    pv[0:65, qlo : qlo + w_],
                    vall[:, m, 65 * h : 65 * h + 65],
                    pt[:, col : col + w_],
                    start=(oi == 0),
                    stop=(oi == len(order) - 1),
                    skip_group_check=True,
                )
            ot = wk_p.tile([65, 512], BF16, name=f"ot{h}")
            # tail-phase epilogues lean on the Act engine, which has no
            # exps left to run there
            if s >= NSB - 3:
                nc.scalar.copy(ot[:], pv[0:65, :])
            else:
                nc.vector.tensor_copy(ot[:], pv[0:65, :])
            st["ots"].append(ot)

        def emit_epi_trans(s, h):
            # all transposes of a superblock are emitted before any recip/
            # scale reads trp: psum tile dependency tracking is coarse, so a
            # later head's transpose (write) would serialize behind earlier
            # heads' scale ops (reads).
            st = blk[s]
            if "rec" not in st:
                st["rec"] = wk_p.tile([128, 16], F32, name="rec")
                st["osbs"] = [
                    wk_p.tile([128, 192], F32, name="osb") for _ in range(4)
                ]
                st["trps"] = []
            if s >= NSB - 3:
                # tail: per-head trp tiles (psum is free of scores there), so
                # one head's scale reads never serialize another's transposes
                trp = ps_p.tile([128, 512], BF16, space="PSUM", tag="ps",
                                name="trp")
                st["trps"].append(trp)
                base = 0
            else:
                if not st["trps"]:
                    st["trps"].append(ps_p.tile(
                        [128, 1536], BF16, space="PSUM", tag="ps", name="trp"
                    ))
                trp = st["trps"][0]
                base = 512 * h
            for g in range(4):
                nc.tensor.transpose(
                    trp[:, base + 66 * g : base + 66 * g + 65],
                    st["ots"][h][0:65, 128 * g : 128 * g + 128],
                    idn_sb[0:65, 0:65],
                )

        def emit_epi_head(s, h):
            # Epilogue: transpose head h's [65, 512] into trp bank h
            # (66-spaced query-quarters; col 64 of each group is the softmax
            # denominator), then scale by the reciprocal row sums.
            st = blk[s]
            rec, osbs = st["rec"], st["osbs"]
            if s >= NSB - 3:
                trp, base = st["trps"][h], 0
            else:
                trp, base = st["trps"][0], 512 * h
            dcol = trp[:, base : base + 264].rearrange(
                "p (i x) -> p i x", x=66
            )[:, :, 64:65]
            nc.vector.reciprocal(
                rec[:, 4 * h : 4 * h + 4].rearrange("p (i x) -> p i x", x=1),
                dcol,
            )
            for g in range(4):
                args = (
                    osbs[g][:, 64 * h : 64 * h + 64],
                    trp[:, base + 66 * g : base + 66 * g + 64],
                    rec[:, 4 * h + g : 4 * h + g + 1],
                )
                # the very last superblock's scales split across engines
                # (nothing queues behind them)
                if s == NSB - 1 and h == 1:
                    nc.scalar.mul(*args)
                else:
                    nc.vector.tensor_scalar_mul(*args)

        def emit_epi_out(s):
            q0 = 512 * s
            st = blk.pop(s)
            for g in range(4):
                if has_kmask:
                    nc.vector.tensor_scalar_mul(
                        st["osbs"][g][:], st["osbs"][g][:],
                        qpad_sb[:, 4 * s + g : 4 * s + g + 1],
                    )
                # tail output DMAs fan out across engine queues: the SP
                # queue's per-DMA issue time (565ns) would otherwise
                # serialize the final drain
                q_ = (nc.sync, nc.scalar, nc.gpsimd, nc.scalar)[g] \
                    if s >= NSB - 3 else nc.sync
                q_.dma_start(
                    out_d[q0 + 128 * g : q0 + 128 * g + 128, 0:192],
                    st["osbs"][g][:],
                )

        # Software pipeline: scores(s) on PE while exp/mask(s-1) drain on
        # Act/Pool, then PV+epilogue(s-1); projection work interleaves.
        # scores(i) needs qk through tile i+1; pv(i-1) needs v through tile
        # i. DMA order front-loads what the first matmuls need: wqk, hst(0),
        # then the rest of the constants.
        wqk_src = wqk_d[:].rearrange("(c p) n -> p c n", p=128)
        nc.sync.dma_start(wqk_sb[:, 0:2, :], wqk_src[:, 0:2, :])
        emit_proj_dma(0, split=True)
        nc.sync.dma_start(wqk_sb[:, 2:6, :], wqk_src[:, 2:6, :])
        nc.sync.dma_start(wv_sb[:], wv_d[:].rearrange("(c p) n -> p c n", p=128))
        emit_proj_qk(0)
        nc.sync.dma_start(msk_sb[:], msk_d[:, :])
        emit_proj_dma(1)
        emit_proj_qk(1)
        emit_proj_v(0)
        emit_proj_dma(2)
        # PV+epilogue trail scores by PVLAG+1 superblocks: the deferred PV
        # work fills PE during the post-projection iterations where scores
        # would otherwise lockstep with the Act engine's exp drain (psum
        # score slots only free once exp'd). trp(p) must allocate after
        # scores(i)'s psum tiles or a later score tile would evict it
        # before its readers.
        PVLAG = 2
        for i in range(NSB + PVLAG + 1):
            p = i - PVLAG - 1
            if i + 3 <= 7:
                emit_proj_dma(i + 3)
            # tail-assist threshold: epilogues running after the last exps
            # drain can borrow the Act engine
            if i + 2 <= 7:
                emit_proj_qk(i + 2)
            if i >= NSB:
                # last iteration: interleave the epilogue chains between the
                # remaining PV heads so the end chain starts sooner
                emit_pv_head(p, 0)
                emit_pv_head(p, 1)
                emit_epi_trans(p, 0)
                emit_pv_head(p, 2)
                emit_epi_head(p, 0)
                emit_epi_trans(p, 1)
                emit_epi_head(p, 1)
                emit_epi_trans(p, 2)
                emit_epi_head(p, 2)
                emit_epi_out(p)
            else:
                for h in range(3):
                    if i < NSB:
                        emit_scores_head(i, h)
                    if p >= 0:
                        emit_pv_head(p, h)
                if p >= 0:
                    for h in range(3):
                        emit_epi_trans(p, h)
                    for h in range(3):
                        emit_epi_head(p, h)
                    emit_epi_out(p)
            if 1 <= i + 1 <= 7:
                emit_proj_v(i + 1)

    nc.compile()
    return nc


_prog_cache = {}


def _get_program(has_bias, has_kmask):
    key = (has_bias, has_kmask)
    if key not in _prog_cache:
        _prog_cache[key] = build_program(has_bias, has_kmask)
    return _prog_cache[key]


def _band_masks():
    """[mL | mL | mR | mR] multiplicative diagonal masks, [128, 512].

    In [key-row r, query-col j] space: mL keeps j <= r (left window edge),
    mR keeps j >= r (right edge); each appears twice so 2-region strided
    mask ops can read consecutive 128-col groups.
    """
    r = np.arange(128)[:, None]
    q = np.arange(128)[None, :]
    mL = (q <= r).astype(np.float32)
    mR = (q >= r).astype(np.float32)
    return np.concatenate([mL, mL, mR, mR], axis=1)


def kernel(hidden_states, attention_mask, Wq, bq, Wk, bk, Wv, bv, _res=[None]):
    hidden_states = np.asarray(hidden_states, np.float32)
    attention_mask = np.asarray(attention_mask, np.float32)
    Wq, Wk, Wv = (np.asarray(w, np.float32) for w in (Wq, Wk, Wv))
    bq, bk, bv = (np.asarray(b_, np.float32) for b_ in (bq, bk, bv))

    scale = 1.0 / np.sqrt(DH)
    has_bias = bool(np.any(bq) or np.any(bk) or np.any(bv))
    has_kmask = bool(np.any(attention_mask < 0))

    hsT = [np.ascontiguousarray(hidden_states[b].T).astype(BF) for b in range(B)]
    masks = _band_masks().astype(BF)
    ident = np.eye(128, dtype=np.float32).astype(BF)
    ident = np.eye(128, dtype=np.float32).astype(BF)
    masked = attention_mask < 0  # [B, S]

    in_maps = []
    for core in range(N_CORES):
        b, h0 = core // 4, (core % 4) * HPC
        sl = slice(h0 * DH, (h0 + HPC) * DH)
        wq = Wq[:, sl] * scale
        wk = Wk[:, sl]
        wqk = np.concatenate(
            [wq[:, 0:128], wk[:, 0:128], wq[:, 128:192], wk[:, 128:192]], axis=1
        )
        m = {
            "hsT": hsT[b],
            "wqk": np.ascontiguousarray(wqk).astype(BF),
            "wv": np.ascontiguousarray(Wv[:, sl]).astype(BF),
            "masks": masks,
            "ident": ident,
        }
        if has_bias:
            bq_s = bq[sl] * scale
            bk_s = bk[sl]
            m["bqk"] = np.concatenate(
                [bq_s[0:128], bk_s[0:128], bq_s[128:192], bk_s[128:192]]
            ).reshape(1, 384).astype(BF)
            m["bv"] = bv[sl].reshape(1, 192).astype(BF)
        if has_kmask:
            keep = (~masked[b]).astype(np.float32).reshape(NKC, 128).T
            m["kpad"] = np.ascontiguousarray(keep)
            m["qpad"] = np.ascontiguousarray(keep)
        in_maps.append(m)

    nc = _get_program(has_bias, has_kmask)
    res = run_bass_kernel_spmd(nc, in_maps, list(range(N_CORES)))
    _res[0] = res

    out = np.empty((B, S, D), np.float32)
    for core in range(N_CORES):
        b, h0 = core // 4, (core % 4) * HPC
        out[b, :, h0 * DH : (h0 + HPC) * DH] = res.results[core]["out"]
    return out
